# revision 13
# baseline (speedup 1.0000x reference)
"""Trainium2 Bass kernel v2 for nn_DescriptorModuleSpecies (gnn_message_passing).

Sharding: one snapshot per NeuronCore (8 cores), full inputs in / full out.

Algebra: D[n] = Q[n]^T @ Q[n][:, :16],  Q[n][d,g] = sum_m r_tilde_d(e) G_g(s_e).
G(s; class) is refit on a shared-knot PL basis phi = [1?, s, relu(s-t_k)] with
class folded into three moment weights {v, v(a+B), v a B} (T-matrix folded
into W3f host-side), so the device computes, per atom-pair column j:
    phi_psum[m*10+b, (nl,d)] = sum_{64 edge rows} bas[row, b] * LT[row, (m,d)]
(3 matmuls x 2 parity halves per column), then Q = W3f^T-contraction (PE),
then D as broadcasted products on DVE. Planes are fp16 (DVE 2x/4x modes);
geometry is f32. Min-image via fused (x+30) mod 20 - 10.
"""

import sys

import numpy as np

try:
    import concourse.bass as bass  # noqa: F401
except Exception:  # pragma: no cover
    sys.path.insert(0, "/opt/trn_rl_repo")

import concourse.bass as bass
import concourse.bacc as bacc
import concourse.mybir as mybir
from concourse.bass_utils import run_bass_kernel_spmd
from concourse.tile import TileContext

F32 = mybir.dt.float32
F16 = mybir.dt.float16
I16 = mybir.dt.int16
AF = mybir.ActivationFunctionType
ALU = mybir.AluOpType

S, N, M = 8, 4096, 64
L = 20.0
JTOT = N // 2               # 2048 atom-pair columns
NCHUNK = 4
JC = JTOT // NCHUNK         # 512 cols per chunk
NI = 16 * JC                # gather num_idxs per core per chunk
NCORES = 8

KNOTS = [0.09, 0.22, 0.44, 0.8, 2.9, 4.3, 5.4, 10.4]
WB = 2 + len(KNOTS)         # basis width: [v, s, relu x 8] = 10
WROWS = 3 * WB              # phi rows (3 m-weights stacked) = 30


# ---------------- host-side weight folding (shared-knot refit) --------------

def _mlp_np(x, params):
    n = len(params)
    for i, (w, b) in enumerate(params):
        x = x @ w + b
        if i < n - 1:
            x = np.maximum(x, 0.0)
    return x


def _exact_G(sv, ci, ws):
    es = [(ws["es1_w"], ws["es1_b"]), (ws["es2_w"], ws["es2_b"])]
    fs = [(ws["fs1_w"], ws["fs1_b"]), (ws["fs2_w"], ws["fs2_b"])]
    CL = [(0, 0), (0, 1), (1, 1)]
    a, b = CL[ci]
    pair = np.array([[a, b]], dtype=np.float64)
    td = _mlp_np(_mlp_np(pair, es) + _mlp_np(pair[:, ::-1], es), fs)[0]
    st = sv[:, None] * td[None, :]
    return _mlp_np(st, [(ws["en1_w"], ws["en1_b"]), (ws["en2_w"], ws["en2_b"]),
                        (ws["en3_w"], ws["en3_b"])])


def _fold_w3f(ws):
    """Fit G_c(s) ~= alpha[c]^T [1, s, relu(s-t)] and fold the class->m-weight
    transform:  e_c = T[c] . (m0, m1, m2) with m = (v, v(a+B), v a B)."""
    g1 = np.linspace(0.0, 0.6, 1200)
    g2 = np.linspace(0.6, 12.2, 1200)
    sv = np.concatenate([g1, g2])
    cols = [np.ones_like(sv), sv] + [np.maximum(sv - t, 0.0) for t in KNOTS]
    P = np.stack(cols, -1)
    lam = 1e-7
    PtP = P.T @ P + lam * np.eye(P.shape[1])
    alphas = []
    for ci in range(3):
        G = _exact_G(sv, ci, ws)
        A = np.linalg.solve(PtP, P.T @ G)
        alphas.append(A)
        resid = np.abs(P @ A - G).max()
        assert resid < 0.05, f"basis refit residual too large: {resid}"
    alpha = np.stack(alphas)                      # [3, WB, 32]
    T = np.array([[1.0, -1.0, 1.0],
                  [0.0, 1.0, -2.0],
                  [0.0, 0.0, 1.0]])
    W3f = np.einsum('cm,cjg->mjg', T, alpha)      # [3, WB, 32]
    # replicate per PE quadrant: w3rep[32q + b, 32m + g] = W3f[m, b, g]
    w3rep = np.zeros((128, 96), np.float16)
    for q in range(4):
        for m in range(3):
            w3rep[32 * q:32 * q + WB, 32 * m:32 * m + 32] = W3f[m]
    return w3rep


# ---------------------------- device program --------------------------------

def _build_program():
    nc = bacc.Bacc("TRN2", target_bir_lowering=False, debug=False,
                   num_devices=NCORES)
    # constants used by scalar-engine activations (bias/scale values)
    consts = [0.0, 1e-12, float(np.pi), 0.5] + [float(-t) for t in KNOTS]
    for v in consts:
        key = (F32, float(v))
        if key in nc.const_aps.aps:
            continue
        t = nc.alloc_sbuf_tensor(f"constf32_{len(nc.const_aps.aps)}", [128, 1], F32)
        nc.gpsimd.memset(t.ap(), float(v))
        nc.const_aps.aps[key] = t.ap()
    nc.all_engine_barrier()

    table = nc.dram_tensor("table", [128, N], F32, kind="ExternalInput")
    geo = nc.dram_tensor("geo", [128, 3 * JTOT], F32, kind="ExternalInput")
    aux = nc.dram_tensor("aux", [128, 3 * JTOT], I16, kind="ExternalInput")
    w3t = nc.dram_tensor("w3f", [128, 96], F16, kind="ExternalInput")
    dout = nc.dram_tensor("dout", [N, 512], F16, kind="ExternalOutput")

    with TileContext(nc) as tc:
        with (
            tc.tile_pool(name="persist", bufs=1) as pp,
            tc.tile_pool(name="geoin", bufs=2) as gp,
            tc.tile_pool(name="gxp", bufs=1) as xp,
            tc.tile_pool(name="edge", bufs=2) as ep,
            tc.tile_pool(name="scratch", bufs=1) as sp,
            tc.tile_pool(name="plane", bufs=2) as lp,
            tc.tile_pool(name="basp", bufs=2) as bpp,
            tc.tile_pool(name="grp", bufs=2) as grpp,
            tc.tile_pool(name="phips", bufs=2, space="PSUM") as psp,
            tc.tile_pool(name="q2ps", bufs=2, space="PSUM") as qsp,
        ):
            tab = pp.tile([128, N], F32)
            nc.sync.dma_start(tab[:], table[:])
            auxs = pp.tile([128, 3 * JTOT], I16)
            nc.sync.dma_start(auxs[:], aux[:])
            w3s = pp.tile([128, 96], F16)
            nc.sync.dma_start(w3s[:], w3t[:])

            vall = auxs[:, JTOT:2 * JTOT].bitcast(F16)
            aivall = auxs[:, 2 * JTOT:3 * JTOT].bitcast(F16)

            for c in range(NCHUNK):
                j0 = c * JC
                stage = gp.tile([128, 8 * 512], F16, tag="stage")
                geoc = gp.tile([128, 3 * JC], F32, tag="geoc")
                nc.sync.dma_start(geoc[:], geo[:, 3 * j0:3 * j0 + 3 * JC])
                vsl = vall[:, j0:j0 + JC]
                aivsl = aivall[:, j0:j0 + JC]

                gx = xp.tile([128, NI], F32, tag="gx")
                nc.gpsimd.ap_gather(out_ap=gx[:], in_ap=tab[:],
                                    idxs_ap=auxs[:, j0:j0 + JC],
                                    channels=128, num_elems=N, d=1, num_idxs=NI)
                # de-interleave components: rows {16k+comp} -> edge planes
                xyzt = ep.tile([128, 4 * JC], F32, tag="xyzt")
                for comp in range(4):
                    src = gx[comp::16, :]
                    src3 = src.rearrange("p (s j) -> p s j", s=16)
                    dst = xyzt[:, comp * JC:(comp + 1) * JC]
                    nc.sync.dma_start(dst, src3)
                XYZ = xyzt[:, 0:3 * JC]
                BJ = xyzt[:, 3 * JC:4 * JC]

                # ---- geometry (f32) ----
                t1 = sp.tile([128, 3 * JC], F32, tag="t1")
                nc.vector.tensor_tensor(out=t1[:], in0=XYZ, in1=geoc[:],
                                        op=ALU.subtract)
                # min image: wrap xj - xi back into [-10, 10] by one period
                nc.vector.add_range_wrap(t1[:], t1[:], shift=0.0,
                                         bound=10.0, period=L)
                sq = sp.tile([128, 3 * JC], F32, tag="sq")
                nc.scalar.activation(sq[:], t1[:], AF.Square)
                r2 = sp.tile([128, JC], F32, tag="r2")
                nc.vector.tensor_tensor(out=r2[:], in0=sq[:, 0:JC],
                                        in1=sq[:, JC:2 * JC], op=ALU.add)
                nc.gpsimd.tensor_tensor(out=r2[:], in0=r2[:],
                                        in1=sq[:, 2 * JC:3 * JC], op=ALU.add)
                r = sp.tile([128, JC], F32, tag="r")
                nc.scalar.activation(r[:], r2[:], AF.Sqrt, bias=1e-12)
                invr = sp.tile([128, JC], F32, tag="invr")
                nc.vector.reciprocal(invr[:], r[:])
                rc = sp.tile([128, JC], F32, tag="rc")
                nc.gpsimd.tensor_scalar(out=rc[:], in0=r[:], scalar1=2.0,
                                        scalar2=6.0, op0=ALU.max, op1=ALU.min)
                swp = sp.tile([128, JC], F32, tag="swp")
                nc.scalar.activation(swp[:], rc[:], AF.Sin,
                                     scale=float(-np.pi / 4), bias=float(np.pi))
                nc.scalar.activation(swp[:], swp[:], AF.Identity,
                                     bias=0.5, scale=0.5)
                vir = sp.tile([128, JC], F32, tag="vir")
                nc.vector.tensor_tensor(out=vir[:], in0=vsl, in1=invr[:],
                                        op=ALU.mult)

                # LT planes (fp16): [s2, rij x3, m1*(s2,rij), m2*(s2,rij)]
                lt = lp.tile([128, 12 * JC], F16, tag="lt")
                s2 = lt[:, 0:JC]
                nc.vector.tensor_tensor(out=s2, in0=swp[:], in1=vir[:],
                                        op=ALU.mult)
                w0 = sp.tile([128, JC], F32, tag="w0")
                nc.vector.tensor_tensor(out=w0[:], in0=s2, in1=invr[:],
                                        op=ALU.mult)
                w0b = bass.AP(w0.tensor, w0[:].offset,
                              [w0[:].ap[0], [0, 3], [1, JC]])
                rij3 = lt[:, JC:4 * JC].rearrange("p (c j) -> p c j", c=3)
                nc.vector.tensor_tensor(out=rij3, in0=t1[:].rearrange(
                    "p (c j) -> p c j", c=3), in1=w0b, op=ALU.mult)

                m1 = sp.tile([128, JC], F16, tag="m1")
                m2 = sp.tile([128, JC], F16, tag="m2")
                nc.gpsimd.tensor_tensor(out=m2[:], in0=aivsl, in1=BJ,
                                        op=ALU.mult)
                nc.gpsimd.tensor_tensor(out=m1[:], in0=vsl, in1=BJ,
                                        op=ALU.mult)
                nc.vector.tensor_tensor(out=m1[:], in0=m1[:], in1=aivsl,
                                        op=ALU.add)
                for q, mw in ((1, m1), (2, m2)):
                    mb = bass.AP(mw.tensor, mw[:].offset,
                                 [mw[:].ap[0], [0, 4], [1, JC]])
                    dst = lt[:, 4 * q * JC:(4 * q + 4) * JC].rearrange(
                        "p (d j) -> p d j", d=4)
                    src = lt[:, 0:4 * JC].rearrange("p (d j) -> p d j", d=4)
                    nc.vector.tensor_tensor(out=dst, in0=src, in1=mb,
                                            op=ALU.mult)

                # basis planes (fp16): [v, s2, relu(s2 - t_k)]
                bas = bpp.tile([128, WB * JC], F16, tag="bas")
                nc.scalar.copy(bas[:, 0:JC], vsl)
                nc.vector.tensor_copy(out=bas[:, JC:2 * JC], in_=s2)
                for k, t in enumerate(KNOTS):
                    nc.scalar.activation(bas[:, (2 + k) * JC:(3 + k) * JC],
                                         s2, AF.Relu, bias=float(-t))

                # ---- moments + Q + D per 4-group batch (128 atoms) ----
                for b in range(JC // 64):          # 8 batches per chunk
                    # phi_a[32*gg + beta, 12*jj + 4*m + d] per parity half
                    phi_e = psp.tile([128, 192], F32, tag="phie")
                    phi_o = psp.tile([128, 192], F32, tag="phio")
                    phab = (phi_e, phi_o)
                    for gg in range(4):
                        for jj in range(16):
                            j = b * 64 + gg * 16 + jj
                            lhsTs = [bas[0:64, j::JC], bas[64:128, j::JC]]
                            rj = lt[:, j:12 * JC:JC]
                            for a in range(2):
                                nc.tensor.matmul(
                                    out=phab[a][32 * gg:32 * gg + WB,
                                               12 * jj:12 * jj + 12],
                                    lhsT=lhsTs[a],
                                    rhs=rj[64 * a:64 * (a + 1), :],
                                    start=True, stop=True,
                                    tile_position=(64 * a, 32 * gg))
                    # interleave parities: phis[., 24*jj + 12*a + 4*m + d]
                    phis = grpp.tile([128, 384], F16, tag="phis")
                    ph = phis[:]
                    for a, pha in ((0, phi_e), (1, phi_o)):
                        dstv = bass.AP(ph.tensor, ph.offset + 12 * a,
                                       [ph.ap[0], [24, 16], [1, 12]])
                        nc.scalar.copy(dstv, pha[:])

                    q2p = qsp.tile([128, 128], F32, tag="q2p")
                    for gg in range(4):
                        pb = phis[32 * gg:32 * gg + WB, :]
                        for d in range(4):
                            for m in range(3):
                                lw = bass.AP(pb.tensor, pb.offset + 4 * m + d,
                                             [pb.ap[0], [12, 32]])
                                nc.tensor.matmul(
                                    out=q2p[32 * gg:32 * gg + 32,
                                            32 * d:32 * d + 32],
                                    lhsT=lw,
                                    rhs=w3s[32 * gg:32 * gg + WB,
                                            32 * m:32 * m + 32],
                                    start=(m == 0), stop=(m == 2),
                                    tile_position=(32 * gg, 32 * gg))
                    q2 = grpp.tile([128, 128], F16, tag="q2")
                    nc.scalar.copy(q2[:], q2p[:])

                    # D[n, 16k+g] = sum_d Q[d, g] * Q[d, k]  (host transposes)
                    q2a = q2[:]
                    tmp = grpp.tile([128, 4 * 512], F16, tag="tmp")
                    in0 = bass.AP(q2a.tensor, q2a.offset,
                                  [q2a.ap[0], [32, 4], [0, 16], [1, 32]])
                    in1 = bass.AP(q2a.tensor, q2a.offset,
                                  [q2a.ap[0], [32, 4], [1, 16], [0, 32]])
                    tmpv = tmp[:].rearrange("p (d k g) -> p d k g", d=4, g=32)
                    if b % 2 == 0:
                        # replicate k-data on Act so the DVE mult runs at 2x
                        krep = grpp.tile([128, 4 * 512], F16, tag="krep")
                        krv = krep[:].rearrange("p (d k g) -> p d k g",
                                                d=4, g=32)
                        nc.scalar.copy(krv, in1)
                        nc.vector.tensor_tensor(out=tmpv, in0=in0, in1=krv,
                                                op=ALU.mult)
                    else:
                        nc.vector.tensor_tensor(out=tmpv, in0=in0, in1=in1,
                                                op=ALU.mult)
                    ta = tmp[:].rearrange("p (e f) -> p e f", e=2)
                    nc.vector.tensor_tensor(
                        out=ta[:, 0, :].rearrange("p (e f) -> p e f", e=2),
                        in0=ta[:, 0, :].rearrange("p (e f) -> p e f", e=2),
                        in1=ta[:, 1, :].rearrange("p (e f) -> p e f", e=2),
                        op=ALU.add)
                    nc.vector.tensor_tensor(
                        out=stage[:, 512 * b:512 * (b + 1)],
                        in0=tmp[:, 0:512], in1=tmp[:, 512:1024], op=ALU.add)

                dst = dout[1024 * c:1024 * (c + 1)].rearrange(
                    "(b p) f -> p b f", p=128)
                src = stage[:].rearrange("p (b f) -> p b f", b=8)
                nc.sync.dma_start(dst, src)

    nc.compile()
    return nc


# ------------------------------ host glue ----------------------------------

def _prep_core(pos, types, neigh):
    comp = np.empty((4, N), np.float32)
    comp[0], comp[1], comp[2] = pos[:, 0], pos[:, 1], pos[:, 2]
    comp[3] = types.astype(np.float32)
    table = np.empty((128, N), np.float32)
    for p in range(4):
        table[p::4] = comp[p]

    nv = neigh.reshape(JTOT, 2, M)
    nq = np.ascontiguousarray(nv.transpose(1, 2, 0).reshape(128, JTOT))
    valid = (nq >= 0)
    nq_cl = np.maximum(nq, 0).astype(np.int16)

    idxw = np.empty((128, JTOT), np.int16)
    for c in range(NCHUNK):
        blk = nq_cl[:, c * JC:(c + 1) * JC]
        for k in range(NCORES):
            stream = blk[16 * k:16 * k + 16, :].reshape(16 * JC)
            idxw[16 * k:16 * k + 16, c * JC:(c + 1) * JC] = \
                stream.reshape(JC, 16).T

    par = pos.reshape(JTOT, 2, 3)

    def repl(x):  # [2, JTOT] -> [128, JTOT]
        return np.ascontiguousarray(
            np.broadcast_to(x[:, None, :], (2, M, JTOT)).reshape(128, JTOT)
        ).astype(np.float32)

    geo = np.empty((128, 3 * JTOT), np.float32)
    for c3 in range(3):
        gr = repl(par[:, :, c3].T)
        for c in range(NCHUNK):
            geo[:, 3 * c * JC + c3 * JC:3 * c * JC + (c3 + 1) * JC] = \
                gr[:, c * JC:(c + 1) * JC]

    vmask = valid.astype(np.float16)
    ai = repl(types.reshape(JTOT, 2).T.astype(np.float32)).astype(np.float16)
    aiv = (ai * vmask).astype(np.float16)
    auxa = np.empty((128, 3 * JTOT), np.int16)
    auxa[:, 0:JTOT] = idxw
    auxa[:, JTOT:2 * JTOT] = vmask.view(np.int16)
    auxa[:, 2 * JTOT:3 * JTOT] = aiv.view(np.int16)
    return dict(table=table, geo=geo, aux=auxa)


_CACHE = {}


def _make_runner(nc):
    """Persistent jitted SPMD executor (run_bass_via_pjrt re-jits per call)."""
    import jax
    from jax.sharding import Mesh, PartitionSpec
    from jax.experimental.shard_map import shard_map
    from concourse import bass2jax

    bass2jax.install_neuronx_cc_hook()

    partition_name = (nc.partition_id_tensor.name
                      if nc.partition_id_tensor else None)
    in_names, out_names, out_avals, zero_outs = [], [], [], []
    for alloc in nc.m.functions[0].allocations:
        if not isinstance(alloc, mybir.MemoryLocationSet):
            continue
        name = alloc.memorylocations[0].name
        if alloc.kind == "ExternalInput":
            if name != partition_name:
                in_names.append(name)
        elif alloc.kind == "ExternalOutput":
            out_names.append(name)
            shape = tuple(alloc.tensor_shape)
            dtype = mybir.dt.np(alloc.dtype)
            out_avals.append(jax.core.ShapedArray(shape, dtype))
            zero_outs.append((shape, dtype))
    n_params = len(in_names)
    all_names = in_names + out_names
    if partition_name is not None:
        all_names = all_names + [partition_name]
    donate = tuple(range(n_params, n_params + len(out_names)))

    def _body(*args):
        operands = list(args)
        if partition_name is not None:
            operands.append(bass2jax.partition_id_tensor())
        outs = bass2jax._bass_exec_p.bind(
            *operands,
            out_avals=tuple(out_avals),
            in_names=tuple(all_names),
            out_names=tuple(out_names),
            lowering_input_output_aliases=(),
            sim_require_finite=True,
            sim_require_nnan=True,
            nc=nc,
        )
        return tuple(outs)

    devices = jax.devices()[:NCORES]
    mesh = Mesh(np.asarray(devices), ("core",))
    n_args = n_params + len(out_names)
    sharded = jax.jit(
        shard_map(_body, mesh=mesh,
                  in_specs=(PartitionSpec("core"),) * n_args,
                  out_specs=(PartitionSpec("core"),) * len(out_names),
                  check_rep=False),
        donate_argnums=donate, keep_unused=True)

    def run(in_maps):
        concat_in = [np.concatenate([m[name] for m in in_maps], axis=0)
                     for name in in_names[:n_params]]
        concat_zeros = [np.zeros((NCORES * s[0], *s[1:]), d)
                        for s, d in zero_outs]
        out_arrs = sharded(*concat_in, *concat_zeros)
        return {name: np.asarray(out_arrs[i]).reshape(
                    NCORES, *out_avals[i].shape)
                for i, name in enumerate(out_names)}

    return run


def kernel(**inputs):
    inputs = {k: np.asarray(v) for k, v in inputs.items()}
    ws = {k: inputs[k].astype(np.float64) for k in
          ("es1_w", "es1_b", "es2_w", "es2_b", "fs1_w", "fs1_b", "fs2_w",
           "fs2_b", "en1_w", "en1_b", "en2_w", "en2_b", "en3_w", "en3_b")}
    key = hash(tuple(ws[k].tobytes() for k in sorted(ws)))
    if key not in _CACHE:
        w3f = _fold_w3f(ws)
        nc = _build_program()
        _CACHE[key] = (w3f, nc, _make_runner(nc))
    w3f, nc, run = _CACHE[key]

    pos = inputs["inputs"].astype(np.float32)
    types = inputs["input_types"].astype(np.int64)
    neigh = inputs["neigh_list"].astype(np.int64)

    in_maps = []
    for s in range(S):
        m = _prep_core(pos[s], types[s], neigh[s])
        m["w3f"] = w3f
        in_maps.append(m)

    outs = run(in_maps)["dout"]
    # device layout is [N, 16 k, 32 g]; transpose to [N, 32, 16]
    out = np.stack([np.ascontiguousarray(
        outs[s].astype(np.float32).reshape(N, 16, 32).transpose(0, 2, 1))
        for s in range(S)], 0)
    return out


# revision 15
# speedup vs baseline: 1.0598x; 1.0598x over previous
"""Trainium2 Bass kernel v2 for nn_DescriptorModuleSpecies (gnn_message_passing).

Sharding: one snapshot per NeuronCore (8 cores), full inputs in / full out.

Algebra: D[n] = Q[n]^T @ Q[n][:, :16],  Q[n][d,g] = sum_m r_tilde_d(e) G_g(s_e).
G(s; class) is refit on a shared-knot PL basis phi = [1?, s, relu(s-t_k)] with
class folded into three moment weights {v, v(a+B), v a B} (T-matrix folded
into W3f host-side), so the device computes, per atom-pair column j:
    phi_psum[m*10+b, (nl,d)] = sum_{64 edge rows} bas[row, b] * LT[row, (m,d)]
(3 matmuls x 2 parity halves per column), then Q = W3f^T-contraction (PE),
then D as broadcasted products on DVE. Planes are fp16 (DVE 2x/4x modes);
geometry is f32. Min-image via fused (x+30) mod 20 - 10.
"""

import sys

import numpy as np

try:
    import concourse.bass as bass  # noqa: F401
except Exception:  # pragma: no cover
    sys.path.insert(0, "/opt/trn_rl_repo")

import concourse.bass as bass
import concourse.bacc as bacc
import concourse.mybir as mybir
from concourse.bass_utils import run_bass_kernel_spmd
from concourse.tile import TileContext

F32 = mybir.dt.float32
F16 = mybir.dt.float16
I16 = mybir.dt.int16
AF = mybir.ActivationFunctionType
ALU = mybir.AluOpType

S, N, M = 8, 4096, 64
L = 20.0
JTOT = N // 2               # 2048 atom-pair columns
NCHUNK = 4
JC = JTOT // NCHUNK         # 512 cols per chunk
NI = 16 * JC                # gather num_idxs per core per chunk
NCORES = 8

KNOTS = [0.09, 0.22, 0.44, 0.8, 2.9, 4.3, 5.4, 10.4]
WB = 2 + len(KNOTS)         # basis width: [v, s, relu x 8] = 10
WROWS = 3 * WB              # phi rows (3 m-weights stacked) = 30


# ---------------- host-side weight folding (shared-knot refit) --------------

def _mlp_np(x, params):
    n = len(params)
    for i, (w, b) in enumerate(params):
        x = x @ w + b
        if i < n - 1:
            x = np.maximum(x, 0.0)
    return x


def _exact_G(sv, ci, ws):
    es = [(ws["es1_w"], ws["es1_b"]), (ws["es2_w"], ws["es2_b"])]
    fs = [(ws["fs1_w"], ws["fs1_b"]), (ws["fs2_w"], ws["fs2_b"])]
    CL = [(0, 0), (0, 1), (1, 1)]
    a, b = CL[ci]
    pair = np.array([[a, b]], dtype=np.float64)
    td = _mlp_np(_mlp_np(pair, es) + _mlp_np(pair[:, ::-1], es), fs)[0]
    st = sv[:, None] * td[None, :]
    return _mlp_np(st, [(ws["en1_w"], ws["en1_b"]), (ws["en2_w"], ws["en2_b"]),
                        (ws["en3_w"], ws["en3_b"])])


def _fold_w3f(ws):
    """Fit G_c(s) ~= alpha[c]^T [1, s, relu(s-t)] and fold the class->m-weight
    transform:  e_c = T[c] . (m0, m1, m2) with m = (v, v(a+B), v a B)."""
    g1 = np.linspace(0.0, 0.6, 1200)
    g2 = np.linspace(0.6, 12.2, 1200)
    sv = np.concatenate([g1, g2])
    cols = [np.ones_like(sv), sv] + [np.maximum(sv - t, 0.0) for t in KNOTS]
    P = np.stack(cols, -1)
    lam = 1e-7
    PtP = P.T @ P + lam * np.eye(P.shape[1])
    alphas = []
    for ci in range(3):
        G = _exact_G(sv, ci, ws)
        A = np.linalg.solve(PtP, P.T @ G)
        alphas.append(A)
        resid = np.abs(P @ A - G).max()
        assert resid < 0.05, f"basis refit residual too large: {resid}"
    alpha = np.stack(alphas)                      # [3, WB, 32]
    T = np.array([[1.0, -1.0, 1.0],
                  [0.0, 1.0, -2.0],
                  [0.0, 0.0, 1.0]])
    W3f = np.einsum('cm,cjg->mjg', T, alpha)      # [3, WB, 32]
    # replicate per PE quadrant: w3rep[32q + b, 32m + g] = W3f[m, b, g]
    w3rep = np.zeros((128, 96), np.float16)
    for q in range(4):
        for m in range(3):
            w3rep[32 * q:32 * q + WB, 32 * m:32 * m + 32] = W3f[m]
    return w3rep


# ---------------------------- device program --------------------------------

def _build_program():
    nc = bacc.Bacc("TRN2", target_bir_lowering=False, debug=False,
                   num_devices=NCORES)
    # constants used by scalar-engine activations (bias/scale values)
    consts = [0.0, 1e-12, float(np.pi), 0.5] + [float(-t) for t in KNOTS]
    for v in consts:
        key = (F32, float(v))
        if key in nc.const_aps.aps:
            continue
        t = nc.alloc_sbuf_tensor(f"constf32_{len(nc.const_aps.aps)}", [128, 1], F32)
        nc.gpsimd.memset(t.ap(), float(v))
        nc.const_aps.aps[key] = t.ap()
    nc.all_engine_barrier()

    table = nc.dram_tensor("table", [128, N], F32, kind="ExternalInput")
    geo = nc.dram_tensor("geo", [128, 3 * JTOT], F32, kind="ExternalInput")
    aux = nc.dram_tensor("aux", [128, 3 * JTOT], I16, kind="ExternalInput")
    w3t = nc.dram_tensor("w3f", [128, 96], F16, kind="ExternalInput")
    dout = nc.dram_tensor("dout", [N, 512], F16, kind="ExternalOutput")

    with TileContext(nc) as tc:
        with (
            tc.tile_pool(name="persist", bufs=1) as pp,
            tc.tile_pool(name="geoin", bufs=2) as gp,
            tc.tile_pool(name="gxp", bufs=1) as xp,
            tc.tile_pool(name="edge", bufs=2) as ep,
            tc.tile_pool(name="scratch", bufs=1) as sp,
            tc.tile_pool(name="plane", bufs=2) as lp,
            tc.tile_pool(name="basp", bufs=2) as bpp,
            tc.tile_pool(name="grp", bufs=2) as grpp,
            tc.tile_pool(name="phips", bufs=2, space="PSUM") as psp,
            tc.tile_pool(name="q2ps", bufs=2, space="PSUM") as qsp,
        ):
            tab = pp.tile([128, N], F32)
            nc.sync.dma_start(tab[:], table[:])
            auxs = pp.tile([128, JTOT], I16)
            nc.sync.dma_start(auxs[:], aux[:, 0:JTOT])
            auxm = pp.tile([128, 2 * JTOT], I16)
            nc.sync.dma_start(auxm[:], aux[:, JTOT:3 * JTOT])
            w3s = pp.tile([128, 96], F16)
            nc.sync.dma_start(w3s[:], w3t[:])

            vall = auxm[:, 0:JTOT].bitcast(F16)
            aivall = auxm[:, JTOT:2 * JTOT].bitcast(F16)

            def geom_block(c):
                """Gather + geometry + LT/basis planes for chunk c."""
                j0 = c * JC
                geoc = gp.tile([128, 3 * JC], F32, tag="geoc")
                nc.sync.dma_start(geoc[:], geo[:, 3 * j0:3 * j0 + 3 * JC])
                vsl = vall[:, j0:j0 + JC]
                aivsl = aivall[:, j0:j0 + JC]

                gx = xp.tile([128, NI], F32, tag="gx")
                nc.gpsimd.ap_gather(out_ap=gx[:], in_ap=tab[:],
                                    idxs_ap=auxs[:, j0:j0 + JC],
                                    channels=128, num_elems=N, d=1, num_idxs=NI)
                # de-interleave components: rows {16k+comp} -> edge planes
                xyzt = ep.tile([128, 4 * JC], F32, tag="xyzt")
                for comp in range(4):
                    src = gx[comp::16, :]
                    src3 = src.rearrange("p (s j) -> p s j", s=16)
                    dst = xyzt[:, comp * JC:(comp + 1) * JC]
                    nc.sync.dma_start(dst, src3)
                XYZ = xyzt[:, 0:3 * JC]
                BJ = xyzt[:, 3 * JC:4 * JC]

                # ---- geometry (f32) ----
                t1 = sp.tile([128, 3 * JC], F32, tag="t1")
                nc.vector.tensor_tensor(out=t1[:], in0=XYZ, in1=geoc[:],
                                        op=ALU.subtract)
                # min image: wrap xj - xi back into [-10, 10] by one period
                nc.vector.add_range_wrap(t1[:], t1[:], shift=0.0,
                                         bound=10.0, period=L)
                sq = sp.tile([128, 3 * JC], F32, tag="sq")
                nc.scalar.activation(sq[:], t1[:], AF.Square)
                r2 = sp.tile([128, JC], F32, tag="r2")
                nc.vector.tensor_tensor(out=r2[:], in0=sq[:, 0:JC],
                                        in1=sq[:, JC:2 * JC], op=ALU.add)
                nc.gpsimd.tensor_tensor(out=r2[:], in0=r2[:],
                                        in1=sq[:, 2 * JC:3 * JC], op=ALU.add)
                r = sp.tile([128, JC], F32, tag="r")
                nc.scalar.activation(r[:], r2[:], AF.Sqrt, bias=1e-12)
                invr = sp.tile([128, JC], F32, tag="invr")
                nc.vector.reciprocal(invr[:], r[:])
                rc = sp.tile([128, JC], F32, tag="rc")
                nc.gpsimd.tensor_scalar(out=rc[:], in0=r[:], scalar1=2.0,
                                        scalar2=6.0, op0=ALU.max, op1=ALU.min)
                swp = sp.tile([128, JC], F32, tag="swp")
                nc.scalar.activation(swp[:], rc[:], AF.Sin,
                                     scale=float(-np.pi / 4), bias=float(np.pi))
                nc.scalar.activation(swp[:], swp[:], AF.Identity,
                                     bias=0.5, scale=0.5)
                vir = sp.tile([128, JC], F32, tag="vir")
                nc.vector.tensor_tensor(out=vir[:], in0=vsl, in1=invr[:],
                                        op=ALU.mult)

                # LT planes (fp16): [s2, rij x3, m1*(s2,rij), m2*(s2,rij)]
                lt = lp.tile([128, 12 * JC], F16, tag="lt")
                s2 = lt[:, 0:JC]
                nc.vector.tensor_tensor(out=s2, in0=swp[:], in1=vir[:],
                                        op=ALU.mult)
                w0 = sp.tile([128, JC], F32, tag="w0")
                nc.vector.tensor_tensor(out=w0[:], in0=s2, in1=invr[:],
                                        op=ALU.mult)
                w0b = bass.AP(w0.tensor, w0[:].offset,
                              [w0[:].ap[0], [0, 3], [1, JC]])
                rij3 = lt[:, JC:4 * JC].rearrange("p (c j) -> p c j", c=3)
                nc.vector.tensor_tensor(out=rij3, in0=t1[:].rearrange(
                    "p (c j) -> p c j", c=3), in1=w0b, op=ALU.mult)

                m1 = sp.tile([128, JC], F16, tag="m1")
                m2 = sp.tile([128, JC], F16, tag="m2")
                nc.gpsimd.tensor_tensor(out=m2[:], in0=aivsl, in1=BJ,
                                        op=ALU.mult)
                nc.gpsimd.tensor_tensor(out=m1[:], in0=vsl, in1=BJ,
                                        op=ALU.mult)
                nc.vector.tensor_tensor(out=m1[:], in0=m1[:], in1=aivsl,
                                        op=ALU.add)
                for q, mw in ((1, m1), (2, m2)):
                    mb = bass.AP(mw.tensor, mw[:].offset,
                                 [mw[:].ap[0], [0, 4], [1, JC]])
                    dst = lt[:, 4 * q * JC:(4 * q + 4) * JC].rearrange(
                        "p (d j) -> p d j", d=4)
                    src = lt[:, 0:4 * JC].rearrange("p (d j) -> p d j", d=4)
                    nc.vector.tensor_tensor(out=dst, in0=src, in1=mb,
                                            op=ALU.mult)

                # basis planes (fp16): [v, s2, relu(s2 - t_k)]
                bas = bpp.tile([128, WB * JC], F16, tag="bas")
                nc.scalar.copy(bas[:, 0:JC], vsl)
                nc.vector.tensor_copy(out=bas[:, JC:2 * JC], in_=s2)
                for k, t in enumerate(KNOTS):
                    nc.scalar.activation(bas[:, (2 + k) * JC:(3 + k) * JC],
                                         s2, AF.Relu, bias=float(-t))
                return lt, bas

            def batch_block(c, lt, bas):
                """Moments + Q + D + output for chunk c (reads lt/bas)."""
                stage = gp.tile([128, 8 * 512], F16, tag="stage")
                for b in range(JC // 64):          # 8 batches per chunk
                    # phi_a[32*gg + beta, 12*jj + 4*m + d] per parity half
                    phi_e = psp.tile([128, 192], F32, tag="phie")
                    phi_o = psp.tile([128, 192], F32, tag="phio")
                    phab = (phi_e, phi_o)
                    for gg in range(4):
                        for jj in range(16):
                            j = b * 64 + gg * 16 + jj
                            lhsTs = [bas[0:64, j::JC], bas[64:128, j::JC]]
                            rj = lt[:, j:12 * JC:JC]
                            for a in range(2):
                                nc.tensor.matmul(
                                    out=phab[a][32 * gg:32 * gg + WB,
                                                12 * jj:12 * jj + 12],
                                    lhsT=lhsTs[a],
                                    rhs=rj[64 * a:64 * (a + 1), :],
                                    start=True, stop=True,
                                    tile_position=(64 * a, 32 * gg))
                    # interleave parities: phis[., 24*jj + 12*a + 4*m + d]
                    phis = grpp.tile([128, 384], F16, tag="phis")
                    ph = phis[:]
                    for a, pha in ((0, phi_e), (1, phi_o)):
                        dstv = bass.AP(ph.tensor, ph.offset + 12 * a,
                                       [ph.ap[0], [24, 16], [1, 12]])
                        nc.scalar.copy(dstv, pha[:])

                    q2p = qsp.tile([128, 128], F32, tag="q2p")
                    for gg in range(4):
                        pb = phis[32 * gg:32 * gg + WB, :]
                        for d in range(4):
                            for m in range(3):
                                lw = bass.AP(pb.tensor, pb.offset + 4 * m + d,
                                             [pb.ap[0], [12, 32]])
                                nc.tensor.matmul(
                                    out=q2p[32 * gg:32 * gg + 32,
                                            32 * d:32 * d + 32],
                                    lhsT=lw,
                                    rhs=w3s[32 * gg:32 * gg + WB,
                                            32 * m:32 * m + 32],
                                    start=(m == 0), stop=(m == 2),
                                    tile_position=(32 * gg, 32 * gg))
                    q2 = grpp.tile([128, 128], F16, tag="q2")
                    nc.scalar.copy(q2[:], q2p[:])

                    # D[n, 16k+g] = sum_d Q[d, g] * Q[d, k] (host transposes)
                    q2a = q2[:]
                    tmp = grpp.tile([128, 4 * 512], F16, tag="tmp")
                    in0 = bass.AP(q2a.tensor, q2a.offset,
                                  [q2a.ap[0], [32, 4], [0, 16], [1, 32]])
                    in1 = bass.AP(q2a.tensor, q2a.offset,
                                  [q2a.ap[0], [32, 4], [1, 16], [0, 32]])
                    tmpv = tmp[:].rearrange("p (d k g) -> p d k g", d=4, g=32)
                    if b % 2 == 0:
                        # replicate k-data on Act so the DVE mult runs at 2x
                        krep = grpp.tile([128, 4 * 512], F16, tag="krep")
                        krv = krep[:].rearrange("p (d k g) -> p d k g",
                                                d=4, g=32)
                        nc.scalar.copy(krv, in1)
                        nc.vector.tensor_tensor(out=tmpv, in0=in0, in1=krv,
                                                op=ALU.mult)
                    else:
                        nc.vector.tensor_tensor(out=tmpv, in0=in0, in1=in1,
                                                op=ALU.mult)
                    ta = tmp[:].rearrange("p (e f) -> p e f", e=2)
                    nc.vector.tensor_tensor(
                        out=ta[:, 0, :].rearrange("p (e f) -> p e f", e=2),
                        in0=ta[:, 0, :].rearrange("p (e f) -> p e f", e=2),
                        in1=ta[:, 1, :].rearrange("p (e f) -> p e f", e=2),
                        op=ALU.add)
                    nc.vector.tensor_tensor(
                        out=stage[:, 512 * b:512 * (b + 1)],
                        in0=tmp[:, 0:512], in1=tmp[:, 512:1024], op=ALU.add)

                dst = dout[1024 * c:1024 * (c + 1)].rearrange(
                    "(b p) f -> p b f", p=128)
                src = stage[:].rearrange("p (b f) -> p b f", b=8)
                nc.sync.dma_start(dst, src)

            # software pipeline: emit chunk c+1's geometry before chunk c's
            # batch phase so the in-order engine queues keep DVE fed across
            # chunk boundaries.
            ctx = geom_block(0)
            for c in range(NCHUNK):
                nxt = geom_block(c + 1) if c + 1 < NCHUNK else None
                batch_block(c, *ctx)
                ctx = nxt

    nc.compile()
    return nc


# ------------------------------ host glue ----------------------------------

def _prep_core(pos, types, neigh):
    comp = np.empty((4, N), np.float32)
    comp[0], comp[1], comp[2] = pos[:, 0], pos[:, 1], pos[:, 2]
    comp[3] = types.astype(np.float32)
    table = np.empty((128, N), np.float32)
    for p in range(4):
        table[p::4] = comp[p]

    nv = neigh.reshape(JTOT, 2, M)
    nq = np.ascontiguousarray(nv.transpose(1, 2, 0).reshape(128, JTOT))
    valid = (nq >= 0)
    nq_cl = np.maximum(nq, 0).astype(np.int16)

    idxw = np.empty((128, JTOT), np.int16)
    for c in range(NCHUNK):
        blk = nq_cl[:, c * JC:(c + 1) * JC]
        for k in range(NCORES):
            stream = blk[16 * k:16 * k + 16, :].reshape(16 * JC)
            idxw[16 * k:16 * k + 16, c * JC:(c + 1) * JC] = \
                stream.reshape(JC, 16).T

    par = pos.reshape(JTOT, 2, 3)

    def repl(x):  # [2, JTOT] -> [128, JTOT]
        return np.ascontiguousarray(
            np.broadcast_to(x[:, None, :], (2, M, JTOT)).reshape(128, JTOT)
        ).astype(np.float32)

    geo = np.empty((128, 3 * JTOT), np.float32)
    for c3 in range(3):
        gr = repl(par[:, :, c3].T)
        for c in range(NCHUNK):
            geo[:, 3 * c * JC + c3 * JC:3 * c * JC + (c3 + 1) * JC] = \
                gr[:, c * JC:(c + 1) * JC]

    vmask = valid.astype(np.float16)
    ai = repl(types.reshape(JTOT, 2).T.astype(np.float32)).astype(np.float16)
    aiv = (ai * vmask).astype(np.float16)
    auxa = np.empty((128, 3 * JTOT), np.int16)
    auxa[:, 0:JTOT] = idxw
    auxa[:, JTOT:2 * JTOT] = vmask.view(np.int16)
    auxa[:, 2 * JTOT:3 * JTOT] = aiv.view(np.int16)
    return dict(table=table, geo=geo, aux=auxa)


_CACHE = {}


def _make_runner(nc):
    """Persistent jitted SPMD executor (run_bass_via_pjrt re-jits per call)."""
    import jax
    from jax.sharding import Mesh, PartitionSpec
    from jax.experimental.shard_map import shard_map
    from concourse import bass2jax

    bass2jax.install_neuronx_cc_hook()

    partition_name = (nc.partition_id_tensor.name
                      if nc.partition_id_tensor else None)
    in_names, out_names, out_avals, zero_outs = [], [], [], []
    for alloc in nc.m.functions[0].allocations:
        if not isinstance(alloc, mybir.MemoryLocationSet):
            continue
        name = alloc.memorylocations[0].name
        if alloc.kind == "ExternalInput":
            if name != partition_name:
                in_names.append(name)
        elif alloc.kind == "ExternalOutput":
            out_names.append(name)
            shape = tuple(alloc.tensor_shape)
            dtype = mybir.dt.np(alloc.dtype)
            out_avals.append(jax.core.ShapedArray(shape, dtype))
            zero_outs.append((shape, dtype))
    n_params = len(in_names)
    all_names = in_names + out_names
    if partition_name is not None:
        all_names = all_names + [partition_name]
    donate = tuple(range(n_params, n_params + len(out_names)))

    def _body(*args):
        operands = list(args)
        if partition_name is not None:
            operands.append(bass2jax.partition_id_tensor())
        outs = bass2jax._bass_exec_p.bind(
            *operands,
            out_avals=tuple(out_avals),
            in_names=tuple(all_names),
            out_names=tuple(out_names),
            lowering_input_output_aliases=(),
            sim_require_finite=True,
            sim_require_nnan=True,
            nc=nc,
        )
        return tuple(outs)

    devices = jax.devices()[:NCORES]
    mesh = Mesh(np.asarray(devices), ("core",))
    n_args = n_params + len(out_names)
    sharded = jax.jit(
        shard_map(_body, mesh=mesh,
                  in_specs=(PartitionSpec("core"),) * n_args,
                  out_specs=(PartitionSpec("core"),) * len(out_names),
                  check_rep=False),
        donate_argnums=donate, keep_unused=True)

    def run(in_maps):
        concat_in = [np.concatenate([m[name] for m in in_maps], axis=0)
                     for name in in_names[:n_params]]
        concat_zeros = [np.zeros((NCORES * s[0], *s[1:]), d)
                        for s, d in zero_outs]
        out_arrs = sharded(*concat_in, *concat_zeros)
        return {name: np.asarray(out_arrs[i]).reshape(
                    NCORES, *out_avals[i].shape)
                for i, name in enumerate(out_names)}

    return run


def kernel(**inputs):
    inputs = {k: np.asarray(v) for k, v in inputs.items()}
    ws = {k: inputs[k].astype(np.float64) for k in
          ("es1_w", "es1_b", "es2_w", "es2_b", "fs1_w", "fs1_b", "fs2_w",
           "fs2_b", "en1_w", "en1_b", "en2_w", "en2_b", "en3_w", "en3_b")}
    key = hash(tuple(ws[k].tobytes() for k in sorted(ws)))
    if key not in _CACHE:
        w3f = _fold_w3f(ws)
        nc = _build_program()
        _CACHE[key] = (w3f, nc, _make_runner(nc))
    w3f, nc, run = _CACHE[key]

    pos = inputs["inputs"].astype(np.float32)
    types = inputs["input_types"].astype(np.int64)
    neigh = inputs["neigh_list"].astype(np.int64)

    in_maps = []
    for s in range(S):
        m = _prep_core(pos[s], types[s], neigh[s])
        m["w3f"] = w3f
        in_maps.append(m)

    outs = run(in_maps)["dout"]
    # device layout is [N, 16 k, 32 g]; transpose to [N, 32, 16]
    out = np.stack([np.ascontiguousarray(
        outs[s].astype(np.float32).reshape(N, 16, 32).transpose(0, 2, 1))
        for s in range(S)], 0)
    return out
